# revision 46
# baseline (speedup 1.0000x reference)
"""GCNConv-variant Trainium2 kernel (8 NeuronCores, SPMD via bass/tile).

Math (from the reference):
    deg  = in-degree of col over all edges               [N]
    dis  = where(deg>0, deg^-1/2, 0)                     [N]
    pp   = sigmoid(p) + 1
    mu   = min(x)
    x1   = (x - mu + 1e-6)^pp                            [N,128]
    agg[i] = sum_{e: row[e]==i} dis[row]*dis[col]*x1[col[e]]
    out  = (agg + 1e-6)^(1/pp) + (1+eps)*x + mu

Distribution (row-sharded destination ranges, 3 launches):
    P0 (one program per core): per-core x-slice min; in-degree of the core's
       owned node range via ones-stationary matmuls (one-hot batches built 8
       at a time with a single broadcast tensor_tensor), accumulated into
       [1, 4*128] PSUM groups; a matmul-transpose pass turns the flat [1, N]
       degree row into the [128, NW] layout; dis.
    host: mu = min of the 8 partial minima (pure gather/reduce of shards).
    P1 (uniform SPMD program): y = dis * (x - mu + 1e-6)^pp for the owned
       node range, emitted bf16 [rows, 128]; batched ACT passes (all Ln,
       then all Exp over 8-window supertiles; ldis applied by a broadcast
       DVE add) so the activation table loads only twice.
    host: concatenate the 8 y-slices, replicate to all cores.
    P2 (one program per core): gather y rows by col via SWDGE dma_gather
       (single-packet mode — multi-packet descriptor emission on the Q7 is
       ~7x slower); ONE broadcast tensor_tensor builds a whole chunk's
       one-hots (tensor_tensor never grabs the DVE/GpSimd shared SBUF port
       pair, so descriptor generation isn't starved); bf16 matmuls
       (P^T @ y) accumulate per-row-window segment sums in PSUM; flushes
       multiply dis in with a PSUM-sourced DVE op and run batched Ln; the
       output transform runs on 4-window supertiles.
"""

import math
import os
import sys
from contextlib import ExitStack

sys.path.insert(0, "/opt/trn_rl_repo")

import numpy as np
import ml_dtypes

import concourse.bass as bass
import concourse.bacc as bacc
import concourse.bass_isa as bass_isa
import concourse.mybir as mybir
import concourse.tile as tile

F32 = mybir.dt.float32
BF16 = mybir.dt.bfloat16
I16 = mybir.dt.int16
I32 = mybir.dt.int32
EPS_NUM = 1e-6
LN_EPS = float(np.log(np.float32(EPS_NUM)))
ALU = mybir.AluOpType
ACT = mybir.ActivationFunctionType


class Cfg:
    def __init__(self, N=100000, E=1600000, D=128, ncores=8, bank_rows=25000,
                 chunk=8, group=2, p0group=4, stg_bufs=10, ppool_bufs=4,
                 psum_bufs=2, single_packet=True):
        assert D == 128
        self.N, self.E, self.D, self.ncores = N, E, D, ncores
        self.rpc_real = N // ncores            # owned rows per core
        assert self.rpc_real * ncores == N
        self.rpc = ((self.rpc_real + 127) // 128) * 128   # padded rows
        self.nwin = self.rpc // 128            # row windows per core
        self.bank_rows = bank_rows             # gather bank size (int16 limit)
        assert bank_rows <= 32768
        self.nbanks = (N + bank_rows - 1) // bank_rows
        self.chunk = chunk                     # gather batches per SWDGE call
        self.group = group                     # windows per P2 PSUM bank tile
        self.p0group = p0group                 # windows per P0 PSUM tile
        self.stg_bufs = stg_bufs               # gather stage ring depth
        self.ppool_bufs = ppool_bufs           # one-hot build pool depth
        self.psum_bufs = psum_bufs             # P2 PSUM group pool depth
        self.single_packet = single_packet     # SWDGE gather packetization


# ----------------------------------------------------------------------------
# host-side planning (pure data movement / layout; no reference math)
# ----------------------------------------------------------------------------

def _wrap_idxs(idx_linear):
    """SWDGE index layout: slot i lives at [i%16, i//16], tiled to 128 parts."""
    n = len(idx_linear)
    assert n % 16 == 0
    a = np.zeros((16, n // 16), np.int16)
    ar = np.arange(n)
    a[ar % 16, ar // 16] = idx_linear.astype(np.int16)
    return np.tile(a, (8, 1))


def _iota_rep(chunk, width=128):
    i = np.arange(width, dtype=np.float32)
    return np.tile(i, (128, chunk)).astype(ml_dtypes.bfloat16)


class Batch:
    __slots__ = ("bank", "win", "sec", "rl", "take", "idx", "chunk_id", "chunk_col")

    def __init__(self, bank, win, sec, rl, take=128):
        self.bank, self.win, self.sec, self.rl = bank, win, sec, rl
        self.take = take


class CorePlan:
    pass


P2W = 64         # one-hot width: 64-row aligned half-windows (no sec)
CALL_CAP = 8     # max batches per gather call (1024 idxs, single-packet max)


def plan_core_p2(rows_local, cols, cfg: Cfg):
    """Plan one core's P2 schedule. rows_local in [0, rpc_real).

    Edges are sorted (group, bank, half-window, row). Every batch targets
    exactly one 64-row half-window: the one-hot is [128, 64] and there is
    exactly one matmul per batch into a [64, 128] PSUM slice. Each (window,
    half) accumulates in its OWN PSUM bank (column-banked group tiles) at
    partition offset half*64, so accumulations never share a PSUM
    zero-region and bank-major order is fine. Gather calls are 8-batch runs
    within a (group, bank) segment; mid-call batch padding gathers row 0 of
    the bank (finite), trailing padding is trimmed via num_idxs.
    """
    bank = cols // cfg.bank_rows
    hwid = rows_local // P2W
    grp = (rows_local // 128) // cfg.group
    order = np.lexsort((rows_local, hwid, bank, grp))
    r = rows_local[order]
    c = (cols - bank * cfg.bank_rows)[order]
    bk = bank[order]
    hw = hwid[order]
    gp = grp[order]

    batches = []     # Batch(bank, win=window, sec=half, rl[128] in [0,64)|-1)
    seg_spans = []   # (bank, [batch indices]) per (grp, bank) segment
    i0 = 0
    nall = len(r)
    while i0 < nall:
        g0, b = int(gp[i0]), int(bk[i0])
        i1 = i0
        while i1 < nall and gp[i1] == g0 and bk[i1] == b:
            i1 += 1
        seg_b0 = len(batches)
        # within the segment, walk half-window runs
        j0 = i0
        while j0 < i1:
            h0 = int(hw[j0])
            j1 = j0
            while j1 < i1 and hw[j1] == h0:
                j1 += 1
            s = j1 - j0
            for k in range((s + 127) // 128):
                a0, a1 = k * 128, min((k + 1) * 128, s)
                take = a1 - a0
                rl = np.concatenate([
                    r[j0 + a0:j0 + a1] - h0 * P2W,
                    np.full(128 - take, -1, np.int64)])
                cb = np.concatenate([
                    c[j0 + a0:j0 + a1], np.zeros(128 - take, np.int64)])
                bt = Batch(b, h0 // 2, h0 % 2, rl.astype(np.float32), take)
                bt.idx = cb
                batches.append(bt)
            j0 = j1
        seg_spans.append((b, list(range(seg_b0, len(batches)))))
        i0 = i1

    # gather calls: runs of up to CALL_CAP batches within a segment
    calls = []       # (bank, slot0, cn, nidx)
    idx_parts = []
    slot0 = 0
    for (b, bis) in seg_spans:
        i = 0
        while i < len(bis):
            j = min(i + CALL_CAP, len(bis))
            cn = j - i
            for k in range(i, j):
                idx_parts.append(batches[bis[k]].idx)
            nidx = (cn - 1) * 128 + batches[bis[j - 1]].take
            calls.append((b, slot0, cn, nidx))
            slot0 += cn * 128
            i = j

    nb = len(batches)
    plan = CorePlan()
    plan.nbatches = nb
    if nb == 0:
        plan.idx_wrapped = np.zeros((128, 8), np.int16)
        plan.row_local = np.zeros((128, 1), np.float32)
        plan.batches = []
        plan.chunks = []
        plan.flushes = {}
        plan.touched_hw = set()
        plan.hw_first = {}
        plan.hw_last = {}
        return plan

    idx_all = np.concatenate(idx_parts)
    plan.idx_wrapped = _wrap_idxs(idx_all)
    rlm = np.stack([bt.rl for bt in batches], axis=1)   # [128, nb]
    plan.row_local = rlm.astype(np.float32)
    plan.batches = batches
    plan.chunks = calls   # (bank, slot0, cn, nidx); batch bi of call = order

    # per-half-window start/stop + per-group flush bookkeeping
    touch = {}           # group -> [batch indices]
    hw_first, hw_last = {}, {}
    bi = 0
    for (b, s0, cn, s) in calls:
        for k in range(cn):
            bt = plan.batches[bi]
            h = bt.win * 2 + bt.sec
            g = bt.win // cfg.group
            touch.setdefault(g, []).append(bi)
            if h not in hw_first:
                hw_first[h] = bi
            hw_last[h] = bi
            bi += 1
    plan.hw_first = hw_first
    plan.hw_last = hw_last
    plan.touched_hw = set(hw_first)
    flushes = {}   # batch_idx -> list of groups to flush
    for g, lst in touch.items():
        flushes.setdefault(max(lst), []).append(g)
    plan.flushes = flushes
    return plan


P0W = 16   # histogram one-hot block width (16-row aligned blocks, no sec)


def plan_core_p0(cols_local, cfg: Cfg):
    """Histogram batching over the core's owned-range cols (sorted).

    Batches are 32-row aligned blocks: each batch's <=128 edges all target
    [blk*32, blk*32+32), so the one-hot is only 32 wide and each batch is a
    single matmul into one [1, 32] PSUM column range.
    """
    v = np.sort(cols_local)
    G = cfg.p0group * (128 // P0W)   # blocks per PSUM group tile [1, G*32]
    batches = []   # blk
    rls = []
    i = 0
    while i < len(v):
        blk = int(v[i]) // P0W
        j = min(i + 128, len(v))
        cut = int(np.searchsorted(v[i:j], (blk + 1) * P0W, "left"))
        j = i + cut
        take = j - i
        rl = np.concatenate([v[i:j] - blk * P0W,
                             np.full(128 - take, -1, np.int64)])
        batches.append(blk)
        rls.append(rl.astype(np.float32))
        i = j

    plan = CorePlan()
    nb = len(batches)
    plan.nbatches = nb
    if nb == 0:
        plan.row_local = np.zeros((128, 1), np.float32)
        plan.batches = []
        plan.chunks = []
        plan.flushes = {}
        plan.first_touch = {}
        plan.last_touch = {}
        return plan
    plan.row_local = np.stack(rls, axis=1)
    plan.batches = batches

    # build chunks of up to cfg.chunk consecutive batches (one DVE build each)
    chunks = []
    i = 0
    while i < nb:
        j = min(i + cfg.chunk, nb)
        chunks.append((i, j - i))
        i = j
    plan.chunks = chunks

    touch = {}
    for bi, blk in enumerate(batches):
        touch.setdefault(blk // G, []).append((bi, blk))
    flushes = {}
    first_touch, last_touch = {}, {}
    for g, lst in touch.items():
        last_bi = max(e[0] for e in lst)
        blks = sorted({e[1] for e in lst})
        flushes.setdefault(last_bi, []).append((g, blks))
        for bi, blk in lst:
            if blk not in first_touch:
                first_touch[blk] = bi
            last_touch[blk] = bi
    plan.flushes = flushes
    plan.first_touch = first_touch
    plan.last_touch = last_touch
    plan.p0_G = G
    return plan


def plan_all(edge_index, cfg: Cfg):
    """Shard edges; returns per-core host data for P0 and P2."""
    row = np.asarray(edge_index[0])
    col = np.asarray(edge_index[1])

    # ---- P0: histogram shards (col buckets by owner range) ----
    owner = col // cfg.rpc_real
    p0_plans = []
    for cr in range(cfg.ncores):
        sel = owner == cr
        loc = (col[sel] - cr * cfg.rpc_real).astype(np.int64)
        p0_plans.append(plan_core_p0(loc, cfg))

    # ---- P2: row shards ----
    rowner = row // cfg.rpc_real
    plans = []
    for cr in range(cfg.ncores):
        sel = rowner == cr
        plans.append(plan_core_p2((row[sel] - cr * cfg.rpc_real).astype(np.int64),
                                  col[sel].astype(np.int64), cfg))
    return p0_plans, plans


# ----------------------------------------------------------------------------
# bass program builders
# ----------------------------------------------------------------------------

def _mk_nc(**kw):
    return bacc.Bacc("TRN2", target_bir_lowering=False, debug=False,
                     enable_partition_id=False, **kw)


def _dyn_loop(nc, tc, cpool, rep_in, ctx):
    rep_sb = cpool.tile([1, 1], I32)
    nc.sync.dma_start(rep_sb[:], rep_in.ap()[:])
    regs = []
    for e in mybir.ALL_ENGINES:
        regs.append(nc.alloc_register(e, f"repreg_{e.name}"))
    nc.regs_load(bass.RegisterHandles(tuple(regs)), rep_sb[0:1, 0:1])
    rep_val = bass.make_scalar_value(
        bass.RegisterHandles(tuple(regs)), min_val=0, max_val=1 << 20)
    ctx.enter_context(tc.For_i(0, rep_val, 1))


def build_p0(cfg: Cfg, plan: CorePlan, rep=0, ablate=None):
    """Per-core program: x-slice min + one-hot matmul degree histogram + dis.

    Histogram: one-hots built 8 batches per DVE tensor_tensor (broadcast
    operand); matmuls keep ONES stationary (1-column weight load, ~free) and
    stream the one-hot as the moving operand -> out[1, 128] accumulated in
    [1, p0group*128] PSUM tiles; a matmul-transpose pass converts the flat
    [1, rpc] degree row to the [128, NW] layout for dis.

    ablate (experiments only): "xmin" = skip histogram; "hist" = skip xmin;
    "build" = builds without matmuls.
    """
    dynamic = rep == -1
    nc = _mk_nc()
    NW = cfg.nwin
    G = getattr(plan, "p0_G", cfg.p0group * (128 // P0W))
    nbat = max(plan.nbatches, 1)
    x = nc.dram_tensor("x_own", [cfg.rpc, 128], F32, kind="ExternalInput")
    rloc = nc.dram_tensor("col_local", [128, nbat], BF16, kind="ExternalInput")
    iota_d = nc.dram_tensor("iota_rep0", [128, cfg.chunk * P0W], BF16,
                            kind="ExternalInput")
    if dynamic:
        rep_in = nc.dram_tensor("rep", [1, 1], I32, kind="ExternalInput")
    xmin = nc.dram_tensor("xmin", [1, 1], F32, kind="ExternalOutput")
    dis = nc.dram_tensor("dis_own", [cfg.rpc], F32, kind="ExternalOutput")

    with tile.TileContext(nc) as tc, ExitStack() as ctx:
        pool = ctx.enter_context(tc.tile_pool(name="p0", bufs=3))
        ppool = ctx.enter_context(tc.tile_pool(name="p0p", bufs=cfg.ppool_bufs))
        psum = ctx.enter_context(tc.tile_pool(name="p0s", bufs=4, space="PSUM"))
        cpool = ctx.enter_context(tc.tile_pool(name="p0c", bufs=1))

        rl_sb = cpool.tile([128, nbat], BF16)
        nc.sync.dma_start(rl_sb[:], rloc.ap()[:])
        iota = cpool.tile([128, cfg.chunk, P0W], BF16)
        nc.sync.dma_start(iota[:], iota_d.ap().rearrange(
            "p (c j) -> p c j", c=cfg.chunk))
        ones_sb = cpool.tile([128, 1], BF16)
        nc.vector.memset(ones_sb[:], 1.0)
        ones1 = cpool.tile([1, 1], F32)
        nc.vector.memset(ones1[:], 1.0)
        deg_flat = cpool.tile([1, NW * 128], F32)
        deg_sb = cpool.tile([128, NW], F32)
        runmin = cpool.tile([128, 1], F32)

        loop_cm = ExitStack()
        if dynamic:
            _dyn_loop(nc, tc, cpool, rep_in, loop_cm)
        ctx.enter_context(loop_cm)

        # ---- x min (8-window supertiles) ----
        do_xmin = ablate not in ("hist", "build")
        do_hist = ablate != "xmin"
        do_mm = ablate not in ("xmin", "build")
        SW = 8
        nt = (NW // SW) * SW * 128
        xt4 = x.ap()[0:nt, :].rearrange("(t w p) f -> t p w f", p=128, w=SW)
        xt3 = x.ap().rearrange("(w p) f -> w p f", p=128)
        nc.vector.memset(runmin[:], 1e30)
        for t in range(NW // SW if do_xmin else 0):
            xt = pool.tile([128, SW, 128], F32)
            nc.sync.dma_start(xt[:], xt4[t])
            red = pool.tile([128, 1], F32)
            nc.vector.tensor_reduce(red[:], xt[:], mybir.AxisListType.XY, ALU.min)
            nc.vector.tensor_tensor(runmin[:], runmin[:], red[:], ALU.min)
        for w in range((NW // SW) * SW, NW if do_xmin else 0):
            xt1 = pool.tile([128, 128], F32, name="xt1", tag="xt1")
            nc.sync.dma_start(xt1[:], xt3[w])
            red1 = pool.tile([128, 1], F32, name="red1", tag="red1")
            nc.vector.tensor_reduce(red1[:], xt1[:], mybir.AxisListType.X, ALU.min)
            nc.vector.tensor_tensor(runmin[:], runmin[:], red1[:], ALU.min)
        negmin = cpool.tile([128, 1], F32)
        nc.vector.tensor_scalar(negmin[:], runmin[:], -1.0, None, ALU.mult)
        allmax = cpool.tile([128, 1], F32)
        nc.gpsimd.partition_all_reduce(allmax[:], negmin[:], 128,
                                       bass_isa.ReduceOp.max)
        minv = cpool.tile([1, 1], F32)
        nc.vector.tensor_scalar(minv[:], allmax[0:1, :], -1.0, None, ALU.mult)
        nc.sync.dma_start(xmin.ap()[:], minv[:])

        # ---- degree histogram: ones stationary, one-hot streamed ----
        live_psum = {}
        for (bt0, cn) in (plan.chunks if do_hist else []):
            P8 = ppool.tile([128, cfg.chunk, P0W], BF16, name="P0P", tag="P0P")
            nc.vector.tensor_tensor(
                P8[:, 0:cn, :], iota[:, 0:cn, :],
                rl_sb[:, bt0:bt0 + cn].broadcast_to([128, cn, P0W]),
                ALU.is_equal)
            for k in range(cn if do_mm else 0):
                bi = bt0 + k
                blk = plan.batches[bi]
                g = blk // G
                if g not in live_psum:
                    live_psum[g] = psum.tile([1, G * P0W], F32,
                                             name="dgrp", tag="dgrp")
                pt = live_psum[g]
                first = plan.first_touch[blk] == bi
                last = plan.last_touch[blk] == bi
                sl = pt[:, (blk % G) * P0W:(blk % G) * P0W + P0W]
                nc.tensor.matmul(sl, ones_sb[:], P8[:, k, :],
                                 start=first, stop=last)
                for (fg, blks) in plan.flushes.get(bi, []):
                    pt = live_psum.pop(fg)
                    runs = []
                    for b in blks:
                        if runs and b == runs[-1][1]:
                            runs[-1][1] = b + 1
                        else:
                            runs.append([b, b + 1])
                    for a, bnd in runs:
                        nc.vector.tensor_copy(
                            deg_flat[:, a * P0W:bnd * P0W],
                            pt[:, (a % G) * P0W:(a % G) * P0W + (bnd - a) * P0W])
        # blocks with no batches at all: zero their deg columns
        if do_hist and do_mm:
            touched_b = set(plan.batches)
            nblk = NW * (128 // P0W)
            runs0 = []
            for b in range(nblk):
                if b not in touched_b:
                    if runs0 and b == runs0[-1][1]:
                        runs0[-1][1] = b + 1
                    else:
                        runs0.append([b, b + 1])
            for a, bnd in runs0:
                nc.vector.memset(deg_flat[:, a * P0W:bnd * P0W], 0.0)
        else:
            nc.vector.memset(deg_flat[:], 0.0)

        # ---- transpose deg_flat [1, NW*128] -> deg_sb [128, NW] via PE ----
        degT = psum.tile([128, NW], F32, name="degT", tag="degT")
        for w in range(NW):
            nc.tensor.matmul(degT[:, w:w + 1],
                             deg_flat[:, w * 128:(w + 1) * 128],
                             ones1[:], start=True, stop=True)
        nc.vector.tensor_copy(deg_sb[:], degT[:])

        # ---- dis = mask * sqrt(1/max(deg,1)) on the [128, NW] layout ----
        mask = cpool.tile([128, NW], F32)
        nc.vector.tensor_scalar(mask[:], deg_sb[:], 0.5, None, ALU.is_ge)
        nc.vector.tensor_scalar(deg_sb[:], deg_sb[:], 1.0, None, ALU.max)
        nc.vector.reciprocal(deg_sb[:], deg_sb[:])
        nc.scalar.activation(deg_sb[:], deg_sb[:], ACT.Sqrt)
        nc.vector.tensor_tensor(deg_sb[:], deg_sb[:], mask[:], ALU.mult)
        nc.sync.dma_start(dis.ap().rearrange("(w p) -> p w", p=128), deg_sb[:])
    nc.compile()
    return nc


def build_p1(cfg: Cfg, rep=0):
    """Uniform SPMD program: y = dis*(x-mu+eps)^pp as bf16 for the owned slice.

    Batched ACT passes over 8-window supertiles (all Ln, then all Exp) so the
    activation table loads only twice; ldis is applied by a broadcast DVE add
    between the passes, keeping every ACT instruction window-uniform.
    """
    dynamic = rep == -1
    nc = _mk_nc()
    x = nc.dram_tensor("x_own", [cfg.rpc, 128], F32, kind="ExternalInput")
    dis = nc.dram_tensor("dis_own", [cfg.rpc], F32, kind="ExternalInput")
    mu = nc.dram_tensor("mu", [1, 1], F32, kind="ExternalInput")
    p_in = nc.dram_tensor("p", [1, 1], F32, kind="ExternalInput")
    if dynamic:
        rep_in = nc.dram_tensor("rep", [1, 1], I32, kind="ExternalInput")
    y = nc.dram_tensor("y_own", [cfg.rpc, 128], BF16, kind="ExternalOutput")

    NW = cfg.nwin
    SW = 8
    with tile.TileContext(nc) as tc, ExitStack() as ctx:
        pool = ctx.enter_context(tc.tile_pool(name="p1", bufs=4))
        cpool = ctx.enter_context(tc.tile_pool(name="p1c", bufs=1))

        # scalars
        psb = cpool.tile([1, 1], F32)
        nc.sync.dma_start(psb[:], p_in.ap()[:])
        sig = cpool.tile([1, 1], F32)
        nc.scalar.activation(sig[:], psb[:], ACT.Sigmoid)
        ppb = cpool.tile([128, 1], F32)
        nc.gpsimd.partition_broadcast(ppb[:], sig[:])
        pp_vec = cpool.tile([128, 1], F32)
        nc.vector.tensor_scalar(pp_vec[:], ppb[:], 1.0, None, ALU.add)
        musb = cpool.tile([1, 1], F32)
        nc.sync.dma_start(musb[:], mu.ap()[:])
        mub = cpool.tile([128, 1], F32)
        nc.gpsimd.partition_broadcast(mub[:], musb[:])
        cvec = cpool.tile([128, 1], F32)   # 1e-6 - mu
        nc.vector.tensor_scalar(cvec[:], mub[:], -1.0, EPS_NUM, ALU.mult, ALU.add)

        dis_sb = cpool.tile([128, NW], F32)
        nc.sync.dma_start(dis_sb[:], dis.ap().rearrange("(w p) -> p w", p=128))
        disc = cpool.tile([128, NW], F32)
        nc.vector.tensor_scalar(disc[:], dis_sb[:], 1e-30, None, ALU.max)
        ldis = cpool.tile([128, NW], F32)
        nc.scalar.activation(ldis[:], disc[:], ACT.Ln)
        # pre-divide by pp: Exp(pp*(t + ldis/pp)) == Exp(pp*t + ldis)
        ippb = cpool.tile([128, 1], F32)
        nc.vector.reciprocal(ippb[:], pp_vec[:])
        nc.vector.tensor_scalar(ldis[:], ldis[:], ippb[:, 0:1], None, ALU.mult)

        tbuf = cpool.tile([128, NW * 128], F32)

        loop_cm = ExitStack()
        if dynamic:
            _dyn_loop(nc, tc, cpool, rep_in, loop_cm)
        ctx.enter_context(loop_cm)

        nt = (NW // SW) * SW * 128
        xt4 = x.ap()[0:nt, :].rearrange("(t w p) f -> t p w f", p=128, w=SW)
        xt3 = x.ap().rearrange("(w p) f -> w p f", p=128)
        yt4 = y.ap()[0:nt, :].rearrange("(t w p) f -> t p w f", p=128, w=SW)
        yt3 = y.ap().rearrange("(w p) f -> w p f", p=128)
        tb3 = tbuf[:].rearrange("p (w f) -> p w f", f=128)

        # pass 1: tbuf = Ln(x - mu + eps) (one table load)
        for t in range(NW // SW):
            xt = pool.tile([128, SW, 128], F32)
            nc.sync.dma_start(xt[:], xt4[t])
            nc.scalar.activation(tb3[:, t * SW:(t + 1) * SW, :],
                                 xt[:], ACT.Ln, bias=cvec[:, 0:1])
        for w in range((NW // SW) * SW, NW):
            xt1 = pool.tile([128, 128], F32, name="xt1", tag="xt1")
            nc.sync.dma_start(xt1[:], xt3[w])
            nc.scalar.activation(tbuf[:, w * 128:(w + 1) * 128], xt1[:],
                                 ACT.Ln, bias=cvec[:, 0:1])
        # gate: pp_gate == pp_vec but depends on every pass-1 window, so the
        # scheduler cannot interleave Exp ops (and their act-table loads)
        # into the Ln pass.
        red = pool.tile([128, 1], F32, name="red_g", tag="red_g")
        nc.vector.tensor_reduce(red[:], tb3[:, :, 0:1],
                                mybir.AxisListType.XY, ALU.max)
        gz = pool.tile([128, 1], F32, name="gz_g", tag="gz_g")
        nc.vector.tensor_scalar(gz[:], red[:], 0.0, None, ALU.mult)
        pp_gate = pool.tile([128, 1], F32, name="ppg", tag="ppg")
        nc.vector.tensor_tensor(pp_gate[:], pp_vec[:], gz[:], ALU.add)
        # pass 2: t2 = tbuf + ldis[w] (broadcast DVE add), y = Exp(pp*t2)
        for t in range(NW // SW):
            t2 = pool.tile([128, SW, 128], F32, name="t2", tag="t2")
            nc.vector.tensor_tensor(
                t2[:], tb3[:, t * SW:(t + 1) * SW, :],
                ldis[:, t * SW:(t + 1) * SW].broadcast_to([128, SW, 128]),
                ALU.add)
            yt = pool.tile([128, SW, 128], BF16, name="yt", tag="yt")
            nc.scalar.activation(yt[:], t2[:], ACT.Exp, scale=pp_gate[:, 0:1])
            nc.sync.dma_start(yt4[t], yt[:])
        for w in range((NW // SW) * SW, NW):
            t2b = pool.tile([128, 128], F32, name="t2b", tag="t2b")
            nc.vector.tensor_tensor(
                t2b[:], tbuf[:, w * 128:(w + 1) * 128],
                ldis[:, w:w + 1].broadcast_to([128, 128]), ALU.add)
            ytb = pool.tile([128, 128], BF16, name="ytb", tag="ytb")
            nc.scalar.activation(ytb[:], t2b[:], ACT.Exp,
                                 scale=pp_gate[:, 0:1])
            nc.sync.dma_start(yt3[w], ytb[:])
    nc.compile()
    return nc


def _gather_lanes(nc):
    """DMASW lane index assigned to each dma_gather, in program order."""
    lanes = []
    for blk in nc.m.functions[0].blocks:
        for inst in blk.instructions:
            if isinstance(inst, mybir.InstDMAGatherAnt):
                lane = None
                si = inst.sync_info
                for u in (si.on_update if si else []):
                    nm = getattr(u, "ant_name", "") or ""
                    if nm.startswith("DMASW"):
                        lane = int(nm[5:].split("_")[0])
                lanes.append(lane)
    return lanes


def build_p2(cfg: Cfg, plan: CorePlan, rep=0):
    """Gather + one-hot matmul segment-sum + output (lane-aligned queues).

    rep=0 -> single body. rep=-1 -> dynamic For_i timing build.

    The tile scheduler assigns SWDGE completion sems round-robin over 8
    lanes; a lane must always be paired with the same queue. Build, read
    back the assigned lanes, and rebuild with queue = lane % 4 until the
    mapping is consistent.
    """
    if rep == -1:
        # dynamic For_i timing build with 4-queue gather rotation. CoreSim's
        # SWDGE lane<->queue lock check rejects this (the inter-iteration
        # semaphore reset is queue-0-attributed), but the reset runs under an
        # all-engine barrier after full DMA quiescence, so zeroing the sems
        # is benign on hardware; test.py verifies the timed program's output
        # across rep counts.
        return _build_p2_q(cfg, plan, 1,
                           [ci % 4 for ci in range(len(plan.chunks))],
                           dynamic=True)
    reps = max(rep, 1)
    queues = [gi % 4 for gi in range(reps * len(plan.chunks))]
    for _ in range(4):
        nc = _build_p2_q(cfg, plan, reps, queues)
        lanes = _gather_lanes(nc)
        assert len(lanes) == len(queues), (len(lanes), len(queues))
        want = [(l % 4 if l is not None else 0) for l in lanes]
        if want == queues:
            return nc
        queues = want
    raise RuntimeError("p2 queue/lane assignment did not converge")


def _build_p2_q(cfg: Cfg, plan: CorePlan, reps, queues, dynamic=False,
                ablate=None):
    do_gath = ablate != "nogath"
    do_comp = ablate != "gath"
    do_mm = do_comp and ablate != "nomm"
    do_tail = do_comp and do_mm and ablate != "notail"
    nc = _mk_nc(num_swdge_queues=4,
                dynamic_dma_scratch_size=32768 if dynamic else 16384)
    NW, G = cfg.nwin, cfg.group
    y = nc.dram_tensor("y_full", [cfg.N, 128], BF16, kind="ExternalInput")
    nbat = max(plan.nbatches, 1)
    gidx = nc.dram_tensor("gth_idx", list(plan.idx_wrapped.shape), I16,
                          kind="ExternalInput")
    rloc = nc.dram_tensor("row_local", [128, nbat], BF16, kind="ExternalInput")
    iota_d = nc.dram_tensor("iota_rep", [128, cfg.chunk * P2W], BF16,
                            kind="ExternalInput")
    x = nc.dram_tensor("x_own", [cfg.rpc, 128], F32, kind="ExternalInput")
    dis = nc.dram_tensor("dis_own", [cfg.rpc], F32, kind="ExternalInput")
    mu = nc.dram_tensor("mu", [1, 1], F32, kind="ExternalInput")
    p_in = nc.dram_tensor("p", [1, 1], F32, kind="ExternalInput")
    eps_in = nc.dram_tensor("eps", [1, 1], F32, kind="ExternalInput")
    if dynamic:
        rep_in = nc.dram_tensor("rep", [1, 1], I32, kind="ExternalInput")
    out = nc.dram_tensor("out_own", [cfg.rpc, 128], F32, kind="ExternalOutput")

    with tile.TileContext(nc) as tc, ExitStack() as ctx:
        cpool = ctx.enter_context(tc.tile_pool(name="c", bufs=1))
        stg = ctx.enter_context(tc.tile_pool(name="stg", bufs=cfg.stg_bufs))
        ppool = ctx.enter_context(tc.tile_pool(name="ph", bufs=cfg.ppool_bufs))
        psum = ctx.enter_context(tc.tile_pool(name="ps", bufs=cfg.psum_bufs,
                                              space="PSUM"))
        opool = ctx.enter_context(tc.tile_pool(name="op", bufs=2))

        # ---- constants / scalars ----
        idx_sb = cpool.tile(list(plan.idx_wrapped.shape), I16)
        nc.sync.dma_start(idx_sb[:], gidx.ap()[:])
        rl_sb = cpool.tile([128, nbat], BF16)
        nc.sync.dma_start(rl_sb[:], rloc.ap()[:])
        iota = cpool.tile([128, cfg.chunk, P2W], BF16)
        nc.sync.dma_start(iota[:], iota_d.ap().rearrange(
            "p (c j) -> p c j", c=cfg.chunk))
        dis_sb = cpool.tile([128, NW], F32)
        nc.sync.dma_start(dis_sb[:], dis.ap().rearrange("(w p) -> p w", p=128))

        psb = cpool.tile([1, 1], F32)
        nc.sync.dma_start(psb[:], p_in.ap()[:])
        sig = cpool.tile([1, 1], F32)
        nc.scalar.activation(sig[:], psb[:], ACT.Sigmoid)
        pp1 = cpool.tile([1, 1], F32)
        nc.vector.tensor_scalar(pp1[:], sig[:], 1.0, None, ALU.add)
        ipps = cpool.tile([1, 1], F32)
        nc.vector.reciprocal(ipps[:], pp1[:])
        ipp_vec = cpool.tile([128, 1], F32)
        nc.gpsimd.partition_broadcast(ipp_vec[:], ipps[:])

        esb = cpool.tile([1, 1], F32)
        nc.sync.dma_start(esb[:], eps_in.ap()[:])
        eb = cpool.tile([128, 1], F32)
        nc.gpsimd.partition_broadcast(eb[:], esb[:])
        oneps = cpool.tile([128, 1], F32)
        nc.vector.tensor_scalar(oneps[:], eb[:], 1.0, None, ALU.add)
        musb = cpool.tile([1, 1], F32)
        nc.sync.dma_start(musb[:], mu.ap()[:])
        mu_vec = cpool.tile([128, 1], F32)
        nc.gpsimd.partition_broadcast(mu_vec[:], musb[:])
        # epsv depends on sig so the Sigmoid table load precedes all Ln ops
        sgb = cpool.tile([128, 1], F32)
        nc.gpsimd.partition_broadcast(sgb[:], sig[:])
        epsv = cpool.tile([128, 1], F32)
        nc.vector.tensor_scalar(epsv[:], sgb[:], 0.0, EPS_NUM,
                                ALU.mult, ALU.add)

        tbuf = cpool.tile([128, NW * 128], F32)
        xbuf = cpool.tile([128, NW, 128], F32)

        loop_cm = ExitStack()
        if dynamic:
            _dyn_loop(nc, tc, cpool, rep_in, loop_cm)
        ctx.enter_context(loop_cm)

        nch = len(plan.chunks)
        touched_hw = plan.touched_hw
        for rep_i in range(reps):
            # prefetch x for the output tail (overlaps the gather phase)
            nc.sync.dma_start(
                xbuf[:], x.ap().rearrange("(w p) f -> p w f", p=128))
            for h in range(NW * 2):
                if h not in touched_hw:
                    nc.vector.memset(
                        tbuf[(h % 2) * 64:(h % 2) * 64 + 64,
                             (h // 2) * 128:(h // 2) * 128 + 128], LN_EPS)

            yap = y.ap()
            live_psum = {}
            stage_c = None
            if not do_gath:
                stage_c = cpool.tile([128, CALL_CAP, 128], BF16,
                                     name="stgc", tag="stgc")
                nc.vector.memset(stage_c[:], 0.5)
            bi = 0
            build_hi = 0       # batches [0, build_hi) have P tiles built
            P_tiles = {}       # build-chunk id -> tile
            for ci, (bank, s0, cn, nidx) in enumerate(plan.chunks):
                if do_gath:
                    stage = stg.tile([128, CALL_CAP, 128], BF16, name="stage",
                                     tag="stage")
                    q = queues[rep_i * nch + ci]
                    nc.gpsimd.dma_gather(
                        stage[:, 0:cn, :],
                        yap[bank * cfg.bank_rows:
                            min((bank + 1) * cfg.bank_rows, cfg.N), :],
                        idx_sb[:, s0 // 16: s0 // 16 + (nidx + 15) // 16],
                        nidx, nidx, 128,
                        queue_num=q, single_packet=cfg.single_packet,
                    )
                else:
                    stage = stage_c
                if not do_comp:
                    # recycle the stage ring with a minimal consumer
                    nc.vector.tensor_tensor(epsv[:], epsv[:],
                                            stage[:, 0, 0:1], ALU.max)
                    bi += cn
                    continue
                # build one-hots 8 batches at a time, decoupled from calls
                while build_hi < bi + cn:
                    b0 = build_hi
                    bn = min(cfg.chunk, plan.nbatches - b0)
                    pt8 = ppool.tile([128, cfg.chunk, P2W], BF16,
                                     name="P8", tag="P8")
                    nc.vector.tensor_tensor(
                        pt8[:, 0:bn, :], iota[:, 0:bn, :],
                        rl_sb[:, b0:b0 + bn].broadcast_to([128, bn, P2W]),
                        ALU.is_equal)
                    P_tiles[b0 // cfg.chunk] = pt8
                    build_hi = b0 + bn
                if not do_mm:
                    nc.vector.tensor_tensor(epsv[:], epsv[:],
                                            P_tiles[bi // cfg.chunk][:, 0, 0:1],
                                            ALU.max)
                    if do_gath:
                        nc.vector.tensor_tensor(epsv[:], epsv[:],
                                                stage[:, 0, 0:1], ALU.max)
                    bi += cn
                    continue
                for k in range(cn):
                    bt = plan.batches[bi]
                    w, half = bt.win, bt.sec
                    h = w * 2 + half
                    key = w // G
                    if key not in live_psum:
                        # one 2KB PSUM bank per (window, half): 512 f32
                        # columns each, G windows x 2 halves
                        live_psum[key] = psum.tile([128, G * 1024], F32,
                                                   name="grp", tag="grp")
                    pt = live_psum[key]
                    first = plan.hw_first[h] == bi
                    last = plan.hw_last[h] == bi
                    c0 = ((w % G) * 2 + half) * 512
                    sl = pt[half * 64:half * 64 + 64, c0:c0 + 128]
                    tk = bt.take
                    Pk = P_tiles[bi // cfg.chunk][0:tk, bi % cfg.chunk, :]
                    nc.tensor.matmul(sl, Pk, stage[0:tk, k, :],
                                     start=first, stop=last)
                    # group complete: Ln from PSUM with scale=dis
                    for fg in plan.flushes.get(bi, []):
                        pt2 = live_psum.pop(fg)
                        for w2 in range(fg * G, min((fg + 1) * G, NW)):
                            for hf in range(2):
                                if (w2 * 2 + hf) not in touched_hw:
                                    continue
                                p0 = hf * 64
                                cc = ((w2 % G) * 2 + hf) * 512
                                nc.scalar.activation(
                                    tbuf[p0:p0 + 64,
                                         w2 * 128:(w2 + 1) * 128],
                                    pt2[p0:p0 + 64, cc:cc + 128],
                                    ACT.Ln, bias=epsv[p0:p0 + 64, 0:1],
                                    scale=dis_sb[p0:p0 + 64, w2:w2 + 1])
                    bi += 1

            # ---- output transform tail (4-window supertiles) ----
            if not do_tail:
                nc.sync.dma_start(out.ap()[0:128, 0:1].rearrange(
                    "p f -> p f"), epsv[:])
                continue
            # gate (see build_p1): keeps the Exp pass out of the Ln pass so
            # the act table loads exactly twice.
            redg = opool.tile([128, 1], F32, name="redg", tag="redg")
            nc.vector.tensor_reduce(
                redg[:],
                tbuf[:].rearrange("p (w f) -> p w f", f=128)[:, :, 0:1],
                mybir.AxisListType.XY, ALU.max)
            gzg = opool.tile([128, 1], F32, name="gzg", tag="gzg")
            nc.vector.tensor_scalar(gzg[:], redg[:], 0.0, None, ALU.mult)
            ippg = opool.tile([128, 1], F32, name="ippg", tag="ippg")
            nc.vector.tensor_tensor(ippg[:], ipp_vec[:], gzg[:], ALU.add)

            TW = 4
            ntw = (NW // TW) * TW * 128
            ot4 = out.ap()[0:ntw, :].rearrange("(t w p) f -> t p w f",
                                               p=128, w=TW)
            ot3 = out.ap().rearrange("(w p) f -> w p f", p=128)
            tb3 = tbuf[:].rearrange("p (w f) -> p w f", f=128)
            for t in range(NW // TW):
                eo = opool.tile([128, TW, 128], F32, name="eo", tag="eo")
                nc.scalar.activation(eo[:], tb3[:, t * TW:(t + 1) * TW, :],
                                     ACT.Exp, scale=ippg[:, 0:1])
                xw = opool.tile([128, TW, 128], F32, name="xw", tag="xw")
                nc.vector.tensor_scalar(xw[:], xbuf[:, t * TW:(t + 1) * TW, :],
                                        oneps[:, 0:1],
                                        mu_vec[:, 0:1], ALU.mult, ALU.add)
                ot = opool.tile([128, TW, 128], F32, name="ot", tag="ot")
                nc.vector.tensor_tensor(ot[:], eo[:], xw[:], ALU.add)
                nc.sync.dma_start(ot4[t], ot[:])
            for w in range((NW // TW) * TW, NW):
                eo1 = opool.tile([128, 128], F32, name="eo1", tag="eo1")
                nc.scalar.activation(eo1[:], tbuf[:, w * 128:(w + 1) * 128],
                                     ACT.Exp, scale=ippg[:, 0:1])
                xw1 = opool.tile([128, 128], F32, name="xw1", tag="xw1")
                nc.vector.tensor_scalar(xw1[:], xbuf[:, w, :], oneps[:, 0:1],
                                        mu_vec[:, 0:1], ALU.mult, ALU.add)
                ot1 = opool.tile([128, 128], F32, name="ot1", tag="ot1")
                nc.vector.tensor_tensor(ot1[:], eo1[:], xw1[:], ALU.add)
                nc.sync.dma_start(ot3[w], ot1[:])
    nc.compile()
    return nc


# ----------------------------------------------------------------------------
# PJRT runners
# ----------------------------------------------------------------------------

def _io_names(nc):
    in_names, out_names, out_avals = [], [], []
    import jax
    for alloc in nc.m.functions[0].allocations:
        if not isinstance(alloc, mybir.MemoryLocationSet):
            continue
        name = alloc.memorylocations[0].name
        if alloc.kind == "ExternalInput":
            if nc.partition_id_tensor is not None and \
                    name == nc.partition_id_tensor.name:
                continue
            in_names.append(name)
        elif alloc.kind == "ExternalOutput":
            out_names.append(name)
            out_avals.append(jax.core.ShapedArray(
                tuple(alloc.tensor_shape), mybir.dt.np(alloc.dtype)))
    return in_names, out_names, out_avals


def run_spmd(nc, in_maps):
    """Uniform program on len(in_maps) cores (the stock shard_map path)."""
    from concourse import bass2jax
    return bass2jax.run_bass_via_pjrt(nc, in_maps, n_cores=len(in_maps))


class SingleRunner:
    """One program pinned to one device; supports async dispatch."""

    def __init__(self, nc, device):
        import jax
        from concourse.bass2jax import _bass_exec_p, install_neuronx_cc_hook
        install_neuronx_cc_hook()
        assert nc.partition_id_tensor is None, "per-core programs must not use partition id"
        self.nc, self.device = nc, device
        self.in_names, self.out_names, self.out_avals = _io_names(nc)
        all_in = tuple(self.in_names + self.out_names)
        out_avals = tuple(self.out_avals)
        out_names = tuple(self.out_names)

        def _body(*args):
            outs = _bass_exec_p.bind(
                *args, out_avals=out_avals, in_names=all_in,
                out_names=out_names, lowering_input_output_aliases=(),
                sim_require_finite=True, sim_require_nnan=True, nc=nc)
            return tuple(outs)

        n_params = len(self.in_names)
        donate = tuple(range(n_params, n_params + len(out_names)))
        self.fn = jax.jit(_body, donate_argnums=donate, keep_unused=True)
        self._dev_inputs = None

    def put_inputs(self, in_map):
        import jax
        self._dev_inputs = [jax.device_put(np.asarray(in_map[n]), self.device)
                            for n in self.in_names]
        jax.block_until_ready(self._dev_inputs)

    def dispatch(self):
        import jax
        import jax.numpy as jnp
        zeros = [jnp.zeros(a.shape, a.dtype, device=self.device)
                 for a in self.out_avals]
        return self.fn(*self._dev_inputs, *zeros)

    def collect(self, futs):
        return {n: np.asarray(f) for n, f in zip(self.out_names, futs)}


# ----------------------------------------------------------------------------
# numpy emulation of the planned P2 schedule (host-side logic check only)
# ----------------------------------------------------------------------------

def emulate_p2(cfg, plan, y_full, x_own, dis_own, mu, pp, eps):
    yv = np.asarray(y_full).astype(np.float32)
    agg = np.zeros((128, cfg.nwin * 128), np.float32)
    bi = 0
    for (bank, s0, cn, nidx) in plan.chunks:
        for k in range(cn):
            bt = plan.batches[bi]
            base = s0 + k * 128
            idx = np.zeros(128, np.int64)
            for i in range(128):
                si = base + i
                idx[i] = plan.idx_wrapped[si % 16, si // 16]
            gl = bank * cfg.bank_rows + idx
            rl = plan.row_local[:, bi].astype(np.float32)
            P = (rl[:, None] == np.arange(P2W)[None, :])
            v = P.T.astype(np.float32) @ yv[gl]       # [64, 128]
            w, half = bt.win, bt.sec
            pr = np.arange(half * 64, half * 64 + 64)
            agg[pr, w * 128:(w + 1) * 128] += v
            bi += 1
    aggn = np.zeros((cfg.rpc, 128), np.float32)
    for w in range(cfg.nwin):
        aggn[w * 128:(w + 1) * 128, :] = agg[:, w * 128:(w + 1) * 128]
    o = np.exp((1.0 / pp) * np.log(dis_own[:, None] * aggn + EPS_NUM))
    return o + (1 + eps) * x_own + mu


# ----------------------------------------------------------------------------
# public entry
# ----------------------------------------------------------------------------

_CACHE = {}


def _setup_jax():
    import jax
    cache = "/tmp/jax_neff_cache"
    os.makedirs(cache, exist_ok=True)
    try:
        jax.config.update("jax_compilation_cache_dir", cache)
        jax.config.update("jax_persistent_cache_min_entry_size_bytes", -1)
        jax.config.update("jax_persistent_cache_min_compile_time_secs", 0.0)
    except Exception:
        pass


def _pad_rows(a, rows, fill):
    if a.shape[0] == rows:
        return np.ascontiguousarray(a)
    out = np.full((rows,) + a.shape[1:], fill, a.dtype)
    out[: a.shape[0]] = a
    return out


def _rl_bf16(row_local):
    return row_local.astype(ml_dtypes.bfloat16)


def kernel(x, eps, p, edge_index):
    import jax
    _setup_jax()
    cfg = Cfg()
    x = np.asarray(x, np.float32)
    eps = np.asarray(eps, np.float32).reshape(1, 1)
    p = np.asarray(p, np.float32).reshape(1, 1)
    edge_index = np.asarray(edge_index)
    assert x.shape == (cfg.N, 128)

    p0_plans, plans = plan_all(edge_index, cfg)
    x_sl = [
        _pad_rows(x[c * cfg.rpc_real:(c + 1) * cfg.rpc_real], cfg.rpc, 1e30)
        for c in range(cfg.ncores)
    ]
    iota = _iota_rep(cfg.chunk, P2W)
    iota0 = _iota_rep(cfg.chunk, P0W)
    devices = jax.devices()[: cfg.ncores]

    # ---- P0 (per-core programs, concurrent) ----
    runners0 = []
    for c in range(cfg.ncores):
        key0 = ("p0", cfg.N, cfg.E, c,
                hash(p0_plans[c].row_local.tobytes()))
        if key0 not in _CACHE:
            _CACHE[key0] = build_p0(cfg, p0_plans[c])
        runners0.append(SingleRunner(_CACHE[key0], devices[c]))
    for c in range(cfg.ncores):
        runners0[c].put_inputs({
            "x_own": x_sl[c], "col_local": _rl_bf16(p0_plans[c].row_local),
            "iota_rep0": iota0,
        })
    futs0 = [r.dispatch() for r in runners0]
    jax.block_until_ready(futs0)
    res0 = [runners0[c].collect(futs0[c]) for c in range(cfg.ncores)]
    mu = np.array(min(float(r["xmin"][0, 0]) for r in res0), np.float32)
    mu = mu.reshape(1, 1)
    dis_sl = [res0[c]["dis_own"] for c in range(cfg.ncores)]

    # ---- P1 ----
    key1 = ("p1", cfg.N)
    if key1 not in _CACHE:
        _CACHE[key1] = build_p1(cfg)
    nc1 = _CACHE[key1]
    in_maps1 = [
        {"x_own": x_sl[c], "dis_own": dis_sl[c], "mu": mu, "p": p}
        for c in range(cfg.ncores)
    ]
    res1 = run_spmd(nc1, in_maps1)
    y_full = np.concatenate(
        [res1[c]["y_own"][: cfg.rpc_real] for c in range(cfg.ncores)], axis=0)

    # ---- P2 ----
    outs = [None] * cfg.ncores
    runners = []
    for c in range(cfg.ncores):
        key2 = ("p2", cfg.N, cfg.E, c,
                hash(plans[c].idx_wrapped.tobytes()),
                hash(plans[c].row_local.tobytes()))
        if key2 not in _CACHE:
            _CACHE[key2] = build_p2(cfg, plans[c])
        runners.append(SingleRunner(_CACHE[key2], devices[c]))
    for c in range(cfg.ncores):
        runners[c].put_inputs({
            "y_full": y_full, "gth_idx": plans[c].idx_wrapped,
            "row_local": _rl_bf16(plans[c].row_local), "iota_rep": iota,
            "x_own": x_sl[c], "dis_own": dis_sl[c],
            "mu": mu, "p": p, "eps": eps,
        })
    futs = [runners[c].dispatch() for c in range(cfg.ncores)]
    jax.block_until_ready(futs)
    for c in range(cfg.ncores):
        outs[c] = runners[c].collect(futs[c])["out_own"][: cfg.rpc_real]
    return np.concatenate(outs, axis=0)
